# revision 90
# baseline (speedup 1.0000x reference)
"""Trainium2 Bass kernel for nn_MultiHeadAttention_37838661877847.

Full-input contract: kernel(**inputs) takes the complete tensors and returns
the complete output. Internally shards across 8 NeuronCores:
  core c -> batch b = c // 2, head-group g = c % 2 (8 heads, 512 dims each).
Each core computes Q/K/V projections for its (batch, head-group) slice
(column-parallel weights), attention for its 8 heads, and a partial output
projection (row-parallel Wo). Host sums core pairs and adds bo + bv @ Wo.T
(the V bias commutes through softmax-weighted averaging, so it is folded
into the output-projection bias on the host).

Engine-level design (per core), built as ONE interleaved instruction stream.
The softmax exp (33.5M elems/core, a ~266us serial floor on the ACT engine
alone) is SPLIT between the ACT engine (true exp, ~2/3 of tiles) and the DVE
(Schraudolph bit-trick exp, ~1/3), so the tensor engine (~283us busy) paces
the kernel instead of the exp stream:

  - Q_T/K_T stored (dl, s) in bf16, host-scaled by CQK each so the score
    PSUM equals 128*log2(e)*s_true. ACT tiles: exp via activation scale;
    DVE tiles: probs bf16 bits = trunc(psum + SCHRAUD_B) written as int16
    through a bitcast AP (~1.8% multiplicative noise, mean-calibrated).
  - scores come out (k, q) per 128-k tile; one [128,512] PSUM bank per
    (head, k-group) so FOUR groups pipeline in flight across the two exp
    engines (spool bufs=4); ets tiles in bf16.
  - attn@V is FLIPPED: out (q, dk+1) accumulating over k with the exp tile
    as the stationary operand -> 65-row bf16 matmuls. V is augmented with a
    ones column per head so the softmax denominator Z lands in column 64;
    normalization is a per-partition reciprocal+scale on DVE.
  - normalized output (q, dl) in bf16 is transposed back to (dl, q) via
    bf16 PE-transpose (bf16 identity: 1 cycle/row), then the output
    projection streams wo.
  - emission interleaves projections / attn@V / transposes / out-proj
    between score+exp groups under a PE-lead budget; startup splits the
    first Q-side DMAs so the serialized DMA resource feeds K then Q with
    minimal PE idle.
"""

import sys

sys.path.insert(0, "/opt/trn_rl_repo")

from collections import deque
from contextlib import ExitStack

import numpy as np

import concourse.bass as bass  # noqa: F401
import concourse.tile as tile
from concourse import bacc, masks, mybir
from concourse.bass_utils import run_bass_kernel_spmd

P = 128
DK = 64  # head dim

# Softmax slope-1 trick: host scales Wq/Wk by CQK each so the score PSUM is
# exactly 128*log2(e) * s_true. The ACT engine then exps with scale=1/SLOPE,
# while DVE/gpsimd compute the same softmax numerator via the Schraudolph
# bit trick: bf16_bits(exp(s)) ~= trunc(psum + SCHRAUD_B) as int16. This
# lets all three engines share the exp workload.
SLOPE = 184.66505644  # 128 / ln(2)
CQK = 4.8044902       # sqrt(SLOPE / 8); 8 = sqrt(dk)
ACT_SCALE = 1.0 / SLOPE
SCHRAUD_B = 16249.0   # 16256 (bf16 bits of 1.0) - 7 mean-ratio calibration

_CACHE = {}


def build_nc(S=2048, D=1024, DL=512, mm_dtype="float32r", n_cores=8,
             repeats=1, phases="ABC"):
    """Build + compile the per-core Bass program (same program on all cores).

    repeats exists only for timing experiments; production uses the default.
    mm_dtype/phases are accepted for test-harness compatibility (the kernel
    uses a fixed mixed f32r/bf16 precision scheme).
    """
    f32 = mybir.dt.float32
    f32r = mybir.dt.float32r
    bf16 = mybir.dt.bfloat16
    Exp = mybir.ActivationFunctionType.Exp

    H = DL // DK          # 8 local heads
    HP = H // 2           # 4 head pairs (one pair per 128-row q/k tile)
    ET = D // P           # 8 contraction tiles for projections
    ST = S // P           # 16 k tiles (and q tiles)
    NDT = DL // P         # 4 dl tiles
    QC = 512              # projection s-chunk
    NQ = S // QC          # 4
    KG = 4                # k-tiles per exp group
    NKG = ST // KG        # 4
    VW = H * (DK + 1)     # 520: v tile width incl. ones columns

    nc = bacc.Bacc("TRN2", target_bir_lowering=False, num_devices=n_cores)

    xqT = nc.dram_tensor("xqT", [D, S], bf16, kind="ExternalInput")
    xkT = nc.dram_tensor("xkT", [D, S], bf16, kind="ExternalInput")
    xvT = nc.dram_tensor("xvT", [D, S], bf16, kind="ExternalInput")
    wqT = nc.dram_tensor("wqT", [D, DL], bf16, kind="ExternalInput")
    wkT = nc.dram_tensor("wkT", [D, DL], bf16, kind="ExternalInput")
    wvT = nc.dram_tensor("wvT", [D, DL], bf16, kind="ExternalInput")
    woT = nc.dram_tensor("woT", [DL, D], bf16, kind="ExternalInput")
    bqd = nc.dram_tensor("bq", [DL, 1], f32, kind="ExternalInput")
    bkd = nc.dram_tensor("bk", [DL, 1], f32, kind="ExternalInput")
    y = nc.dram_tensor("y", [S, D], f32, kind="ExternalOutput")

    def mm(out, lhsT, rhs, start, stop):
        nc.tensor.matmul(out, lhsT=lhsT, rhs=rhs, start=start, stop=stop)

    with tile.TileContext(nc) as tc, ExitStack() as top:
        top.enter_context(
            nc.allow_low_precision(
                reason="attention path in bf16; PSUM accumulation stays fp32"
            )
        )
        persist = top.enter_context(tc.tile_pool(name="persist", bufs=1))
        qt = [persist.tile([P, S], bf16, tag=f"qt{i}", name=f"qt{i}") for i in range(NDT)]
        kt = [persist.tile([P, S], bf16, tag=f"kt{i}", name=f"kt{i}") for i in range(NDT)]
        vt = [persist.tile([P, VW], bf16, tag=f"vt{i}", name=f"vt{i}") for i in range(ST)]
        oaT = [persist.tile([P, S], bf16, tag=f"oaT{i}", name=f"oaT{i}") for i in range(NDT)]
        ident = persist.tile([P, P], bf16, tag="ident", name="ident")
        bq_t = persist.tile([P, NDT], f32, tag="bq", name="bq")
        bk_t = persist.tile([P, NDT], f32, tag="bk", name="bk")

        masks.make_identity(nc, ident[:])
        warm = persist.tile([P, P], bf16, tag="warm", name="warm")
        nc.vector.memset(warm[:], 0.0)
        # vt ones-columns are memset inside slot 0 (below) so the head's
        # K/Q projection evacuations reach the DVE queue first.

        # PSUM: scores/exp 4x[128,512] (4 banks) + attn@V accum 2x[128,260]
        # (2 banks) + generic matmul 2x[128,512] (2 banks) = 8 banks.
        spool = top.enter_context(tc.tile_pool(name="spool", bufs=4, space="PSUM"))
        acpool = top.enter_context(tc.tile_pool(name="acpool", bufs=2, space="PSUM"))
        gpool = top.enter_context(tc.tile_pool(name="gpool", bufs=2, space="PSUM"))

        # weight/x pools for Q (live through all Q chunks); wide layouts:
        # w tiles hold all ET contraction blocks side by side (one DMA each).
        wqp = top.enter_context(tc.tile_pool(name="wqp", bufs=1))
        wq = wqp.tile([P, ET * DL], bf16, tag="wq", name="wq")
        xqp = top.enter_context(tc.tile_pool(name="xqp", bufs=1))

        # long-lived attention pools (opened before any scoped pool so that
        # mid-stream pool closes stay LIFO)
        etsp = top.enter_context(tc.tile_pool(name="etsp", bufs=2))
        oasp = top.enter_context(tc.tile_pool(name="oasp", bufs=4))
        yvp = top.enter_context(tc.tile_pool(name="yvp", bufs=2))
        rcp = top.enter_context(tc.tile_pool(name="rcp", bufs=4))

        for _rep in range(repeats):
            # ---------------- pools for K and Q chunk streams -------------
            vstate = {}
            s3 = ExitStack()
            s2 = ExitStack()
            vxa = s2.enter_context(tc.tile_pool(name="vxa", bufs=1))
            vstate["wv"] = vxa.tile([P, ET * DL], bf16, tag="wv", name="wv")
            vstate["xv0"] = vxa.tile([P, ET * (S // 2)], bf16, tag="xv0",
                                     name="xv0")
            s1 = ExitStack()
            kx = s1.enter_context(tc.tile_pool(name="kx", bufs=2))
            wkp = s1.enter_context(tc.tile_pool(name="wkp", bufs=1))
            wk = wkp.tile([P, ET * DL], bf16, tag="wk", name="wk")

            def load_xk(c, eng=None):
                xkc = kx.tile([P, ET * QC], bf16, tag="xk", name="xk")
                (eng or nc.sync).dma_start(
                    out=xkc[:].rearrange("p (e s) -> p e s", e=ET),
                    in_=xkT[:, c * QC : (c + 1) * QC].rearrange(
                        "(e p) s -> p e s", p=P),
                )
                vstate["xk"] = xkc

            def load_xq(c, eng):
                xqc = xqp.tile([P, ET * QC], bf16, tag="xq", name="xq")
                eng.dma_start(
                    out=xqc[:].rearrange("p (e s) -> p e s", e=ET),
                    in_=xqT[:, c * QC : (c + 1) * QC].rearrange(
                        "(e p) s -> p e s", p=P),
                )
                vstate["xq"] = xqc

            def proj_piece(c, dch, w, xkey, bias, out_tiles):
                """One (chunk, dl-tile) projection: out (dl 128, s 512) + bias."""
                x = vstate[xkey]
                gp = gpool.tile([P, QC], f32, tag="gp", name="gp")
                for e in range(ET):
                    mm(gp[:], w[:, e * DL + dch * P : e * DL + (dch + 1) * P],
                       x[:, e * QC : (e + 1) * QC], e == 0, e == ET - 1)
                nc.vector.tensor_scalar_add(
                    out_tiles[dch][:, c * QC : (c + 1) * QC], gp[:],
                    bias[:, dch : dch + 1]
                )

            # --- V pools: wv + the first s-half of xv preload alongside the
            # K pool (slot 0); the second s-half lands in the space the K pool
            # frees. V projection runs head-half-major so attn@V for heads 0-3
            # unblocks as early as possible.
            SH = S // 2

            def load_wv():
                nc.sync.dma_start(
                    out=vstate["wv"][:].rearrange("p (e d) -> p e d", e=ET),
                    in_=wvT[:].rearrange("(e p) d -> p e d", p=P),
                )

            def load_xv0():
                nc.sync.dma_start(
                    out=vstate["xv0"][:].rearrange("p (e s) -> p e s", e=ET),
                    in_=xvT[:, 0:SH].rearrange("(e p) s -> p e s", p=P),
                )

            def open_vx():
                vxb = s2.enter_context(tc.tile_pool(name="vxb", bufs=1))
                vstate["xv1"] = vxb.tile([P, ET * SH], bf16, tag="xv1", name="xv1")
                nc.sync.dma_start(
                    out=vstate["xv1"][:].rearrange("p (e s) -> p e s", e=ET),
                    in_=xvT[:, SH:S].rearrange("(e p) s -> p e s", p=P),
                )

            def vproj_piece(st, qtr):
                """V projection for (s-tile st, head pair qtr): 2 heads.
                Quarters 0-1 run before attn@V starts, so odd s-tiles borrow
                the idle attn@V accumulator banks for deeper pipelining."""
                Q4 = DL // 4
                if qtr < 2 and st % 2 == 1:
                    gp = acpool.tile([P, 512], f32, tag="ac", name="ac")
                else:
                    gp = gpool.tile([P, QC], f32, tag="gp", name="gp")
                wv = vstate["wv"]
                xv = vstate["xv0"] if st < ST // 2 else vstate["xv1"]
                stl = st % (ST // 2)
                for e in range(ET):
                    mm(gp[:, 0:Q4],
                       xv[:, e * SH + stl * P : e * SH + (stl + 1) * P],
                       wv[:, e * DL + qtr * Q4 : e * DL + (qtr + 1) * Q4],
                       e == 0, e == ET - 1)
                nc.vector.tensor_copy(
                    vt[st][:].rearrange("p (h c) -> p h c", h=H)
                    [:, qtr * 2 : (qtr + 1) * 2, 0:DK],
                    gp[:, 0:Q4].rearrange("p (h c) -> p h c", h=2),
                )

            state = {
                "ets": {},    # (qt_idx, hp, kg) -> tile  (live window)
                "ac": {},     # (qt_idx, hgrp) -> tile
                "oas": {},    # qt_idx -> tile
                "wo": None,
            }

            exp_eng = {}  # (qi, h, kg) -> 0 act / 1 dve / 2 pool

            def sc_use(qi, h, kg):
                """Scores + exp for (q-tile qi, head h, k-group kg).

                One PSUM bank per group so four groups pipeline in flight;
                the exp may run on ACT (true exp) or DVE/gpsimd (Schraudolph
                bits via int16 add, see module docstring constants)."""
                hp, r0 = divmod(h, 2)
                r0 *= DK
                sc = spool.tile([P, KG * P], f32, tag="sc", name="sc")
                for ktl in range(KG):
                    ki = kg * KG + ktl
                    mm(
                        sc[:, ktl * P : (ktl + 1) * P],
                        kt[hp][r0 : r0 + DK, ki * P : (ki + 1) * P],
                        qt[hp][r0 : r0 + DK, qi * P : (qi + 1) * P],
                        True,
                        True,
                    )
                et = etsp.tile([P, KG * P], bf16, tag=f"et{h}_{kg}",
                               name=f"et{h}_{kg}")
                eng = exp_eng.get((qi, h, kg), 0)
                if eng == 0:
                    nc.scalar.activation(et[:], sc[:], Exp, scale=ACT_SCALE)
                else:
                    eo = nc.vector if eng == 1 else nc.gpsimd
                    eo.tensor_scalar_add(
                        et[:].bitcast(mybir.dt.int16), sc[:], SCHRAUD_B
                    )
                state["ets"][(qi, h, kg)] = et

            def attnv_piece(qi, h, kg):
                """attn@V for (q-tile qi, head h, k-group kg): 4 x 65-row mms."""
                hgrp, hidx = divmod(h, 4)
                key = (qi, hgrp)
                if key not in state["ac"]:
                    # padded to a full 2KB bank; cols 0-259 used (4 heads x 65)
                    state["ac"][key] = acpool.tile([P, 512], f32, tag="ac", name="ac")
                ac = state["ac"][key]
                et = state["ets"][(qi, h, kg)]
                for ktl in range(KG):
                    ki = kg * KG + ktl
                    mm(
                        ac[:, hidx * (DK + 1) : (hidx + 1) * (DK + 1)],
                        et[:, ktl * P : (ktl + 1) * P],
                        vt[ki][:, h * (DK + 1) : (h + 1) * (DK + 1)],
                        ki == 0,
                        ki == ST - 1,
                    )
                del state["ets"][(qi, h, kg)]

            def norm_piece(qi, hgrp, on_act=False):
                """Normalize 4 heads: oa_s[:, hgrp*256:+256] = num * (1/Z).
                on_act moves the scale-muls to the ACT engine (used in the
                tail where ACT is idle but DVE still drains exps)."""
                if qi not in state["oas"]:
                    state["oas"][qi] = oasp.tile([P, DL], bf16, tag="oas", name="oas")
                oas = state["oas"][qi]
                ac = state["ac"].pop((qi, hgrp))
                acr = ac[:, 0 : 4 * (DK + 1)].rearrange("p (h c) -> p h c", h=4)
                rc = rcp.tile([P, 4], f32, tag="rc", name="rc")
                nc.vector.reciprocal(rc[:], acr[:, :, DK])
                for hh in range(4):
                    dst = oas[:, hgrp * 4 * DK + hh * DK
                              : hgrp * 4 * DK + (hh + 1) * DK]
                    if on_act:
                        nc.scalar.mul(dst, acr[:, hh, 0:DK], rc[:, hh : hh + 1])
                    else:
                        nc.vector.tensor_scalar_mul(
                            dst, acr[:, hh, 0:DK], rc[:, hh : hh + 1])

            def transp_qt(qi, half=None, via_pe=True, copy_act=False):
                """Transpose oa_s (q, dl) -> oaT (dl, q) for one q-tile.
                half=0 covers dl-tiles 0-1 (ready after the hgrp-0 norm),
                half=1 covers 2-3; None does both. Steady-state q-tiles use
                the DMA xbar transpose (off the PE/DVE critical engines;
                c_piece consumes oaT two slots later so the ~2.5us DMA
                latency is free); the final q-tile stays on the PE path to
                keep the tail chain short."""
                rng = (range(NDT) if half is None
                       else range(2 * half, 2 * half + 2))
                oas = state["oas"][qi]
                if not via_pe:
                    for dlb in rng:
                        nc.sync.dma_start_transpose(
                            oaT[dlb][:, qi * P : (qi + 1) * P],
                            oas[:, dlb * P : (dlb + 1) * P],
                        )
                else:
                    sc = acpool.tile([P, 512], bf16, tag="ac", name="ac")
                    for dlb in rng:
                        nc.tensor.transpose(
                            sc[:, dlb * P : (dlb + 1) * P],
                            oas[:, dlb * P : (dlb + 1) * P],
                            ident[:],
                        )
                    for dlb in rng:
                        (nc.scalar.copy if copy_act else nc.vector.tensor_copy)(
                            oaT[dlb][:, qi * P : (qi + 1) * P],
                            sc[:, dlb * P : (dlb + 1) * P],
                        )
                if half != 0:
                    del state["oas"][qi]

            def load_wo():
                wop = s3.enter_context(tc.tile_pool(name="wop", bufs=1))
                wo = wop.tile([P, NDT * D], bf16, tag="wo", name="wo")
                nc.gpsimd.dma_start(
                    out=wo[:].rearrange("p (i d) -> p i d", i=NDT),
                    in_=woT[:].rearrange("(i p) d -> p i d", p=P),
                )
                state["wo"] = wo

            def c_piece(st, fc, copy_eng=None):
                """Output projection for (s-tile st, f-chunk fc). The PSUM
                evacuation runs on DVE mid-stream; the tail passes the ACT
                engine (idle by then) so the last stores don't queue behind
                DVE's final exp wave."""
                wo = state["wo"]
                gp = gpool.tile([P, QC], f32, tag="gp", name="gp")
                for dl in range(NDT):
                    mm(gp[:], oaT[dl][:, st * P : (st + 1) * P],
                       wo[:, dl * D + fc * QC : dl * D + (fc + 1) * QC],
                       dl == 0, dl == NDT - 1)
                yv = yvp.tile([P, QC], f32, tag="yv", name="yv")
                if copy_eng is None:
                    nc.vector.tensor_copy(yv[:], gp[:])
                else:
                    copy_eng.copy(yv[:], gp[:])
                nc.sync.dma_start(
                    out=y[st * P : (st + 1) * P, fc * QC : (fc + 1) * QC], in_=yv[:]
                )

            # ------------- interleaved emission ---------------------------
            # One FIFO of side pieces per slot, drained between score+exp
            # groups under a PE-lead budget, force-drained at slot end (and at
            # the MID marker before the hp2/hp3 half). Estimated PE ns/piece.
            EXP_NS, SC_NS = 610.0, 215.0
            COST = {}

            def piece_cost(p):
                fn = p[0]
                if fn == proj_piece:
                    return 1750.0
                if fn == qproj_rest:
                    return 1350.0
                if fn == vproj_piece:
                    return 450.0
                if fn == attnv_piece:
                    return 160.0
                if fn == transp_qt:
                    return 520.0
                if fn == c_piece:
                    return 900.0
                return 0.0

            def run_piece(p):
                if p[0] == "loadxk":
                    load_xk(p[1])
                elif p[0] == "loadxq":
                    load_xq(p[1], nc.gpsimd)
                elif p[0] == "loadwo":
                    load_wo()
                elif p[0] == "closes1":
                    s1.close()
                elif p[0] == "openvx":
                    open_vx()
                elif p[0] == "memset":
                    # only the per-head ones-columns (col DK of each head)
                    nc.vector.memset(
                        vt[p[1]][:].rearrange("p (h c) -> p h c", h=H)[:, :, DK],
                        1.0,
                    )
                elif p[0] == "loadwv":
                    load_wv()
                elif p[0] == "loadxv0":
                    load_xv0()
                elif p[0] == "closes2":
                    s2.close()
                elif p[0] == "HPM":
                    pass
                else:
                    p[0](*p[1:])

            def attnv_hp(qi, hp):
                out = []
                for h in (2 * hp, 2 * hp + 1):
                    for kg in range(NKG):
                        out.append((attnv_piece, qi, h, kg))
                return out

            slot_inter = [[] for _ in range(ST)]
            slot_markers = [set() for _ in range(ST)]
            slot_inter[0] += [("memset", i) for i in range(ST)]
            slot_inter[0] += [("loadwv",), ("loadxv0",)]
            slot_inter[0] += [(vproj_piece, st, 0) for st in range(ST // 2)]
            slot_inter[1] += [(vproj_piece, st, 0) for st in range(ST // 2, ST)]
            slot_inter[1] += attnv_hp(0, 0)
            slot_inter[1] += [(vproj_piece, st, 1) for st in range(ST)]
            slot_inter[1] += attnv_hp(0, 1) + [(norm_piece, 0, 0)]
            slot_inter[2] += [(vproj_piece, st, 2) for st in range(ST)]
            slot_inter[2] += attnv_hp(0, 2) + [("HPM", 2)]
            slot_inter[2] += [(vproj_piece, st, 3) for st in range(ST)]
            slot_inter[2] += attnv_hp(0, 3) + [(norm_piece, 0, 1), ("HPM", 3)]
            slot_inter[2] += [("closes2",), ("loadwo",)]
            slot_markers[2] = {2, 3}
            slot_inter[3] += (attnv_hp(1, 0) + [("HPM", 0)]
                              + attnv_hp(1, 1) + [(norm_piece, 1, 0), ("HPM", 1)]
                              + attnv_hp(1, 2) + [("HPM", 2)]
                              + attnv_hp(1, 3) + [(norm_piece, 1, 1), ("HPM", 3)])
            slot_markers[3] = {0, 1, 2, 3}
            slot_inter[3] += [(transp_qt, 0), (transp_qt, 1)]
            for j in range(3, ST):
                slot_inter[j] += (attnv_hp(j - 1, 0) + attnv_hp(j - 1, 1)
                                  + [(norm_piece, j - 1, 0)]
                                  + attnv_hp(j - 1, 2) + attnv_hp(j - 1, 3)
                                  + [(norm_piece, j - 1, 1)])
                if j - 1 >= 2:
                    slot_inter[j].append((transp_qt, j - 1))
            # out-projection: early s-tiles deferred to late slots (the early
            # slots carry the V/K/Q overload), the rest two slots after their
            # transpose.
            for st in range(0, 5):
                slot_inter[11 + st] += [(c_piece, st, 0), (c_piece, st, 1)]
            for st in range(5, 14):
                slot_inter[st + 2] += [(c_piece, st, 0), (c_piece, st, 1)]
            # Q chunk c: DMA early, project each dl-tile just before the
            # first score group of slot 4c that needs it.
            def qproj_mini(c, dch):
                """Q projection for q-tile 4c only (slot 4c's own columns)."""
                x = vstate["xq"]
                gp = gpool.tile([P, QC], f32, tag="gp", name="gp")
                for e in range(ET):
                    mm(gp[:, 0:P],
                       wq[:, e * DL + dch * P : e * DL + (dch + 1) * P],
                       x[:, e * QC : e * QC + P], e == 0, e == ET - 1)
                nc.vector.tensor_scalar_add(
                    qt[dch][:, 4 * c * P : (4 * c + 1) * P], gp[:, 0:P],
                    bq_t[:, dch : dch + 1])

            def qproj_rest(c, dch):
                """Q projection for q-tiles 4c+1..4c+3 (needed next slot)."""
                x = vstate["xq"]
                gp = gpool.tile([P, QC], f32, tag="gp", name="gp")
                for e in range(ET):
                    mm(gp[:, 0 : 3 * P],
                       wq[:, e * DL + dch * P : e * DL + (dch + 1) * P],
                       x[:, e * QC + P : (e + 1) * QC], e == 0, e == ET - 1)
                nc.vector.tensor_scalar_add(
                    qt[dch][:, (4 * c + 1) * P : (4 * c + 4) * P],
                    gp[:, 0 : 3 * P], bq_t[:, dch : dch + 1])

            slot_hp = [[[] for _ in range(HP)] for _ in range(ST)]
            for c in (1, 2, 3):
                slot_inter[4 * c - 3].append(("loadxq", c))
                for dch in range(NDT):
                    slot_hp[4 * c][dch].append((qproj_mini, c, dch))
                slot_inter[4 * c] += [(qproj_rest, c, dch)
                                      for dch in range(NDT)]

            # ---------------- head + slot loop ----------------------------
            # initial loads: tiny biases first, then the four big transfers in
            # the order the serialized DMA resource should grant them — wk and
            # xk0 feed the first K projections; the Q side is split so q-tile-0
            # columns (xq0a, wq dch-0) land early enough for the first scores.
            nc.scalar.dma_start(
                out=wk[:].rearrange("p (e d) -> p e d", e=ET),
                in_=wkT[:].rearrange("(e p) d -> p e d", p=P))
            load_xk(0)
            xqc0 = xqp.tile([P, ET * QC], bf16, tag="xq", name="xq")
            vstate["xq"] = xqc0
            nc.scalar.dma_start(
                out=xqc0[:].rearrange("p (e s) -> p e s", e=ET)[:, :, 0:P],
                in_=xqT[:, 0:P].rearrange("(e p) s -> p e s", p=P))
            wqr = wq[:].rearrange("p (e d) -> p e d", e=ET)
            nc.sync.dma_start(
                out=wqr[:, :, 0:P],
                in_=wqT[:, 0:P].rearrange("(e p) d -> p e d", p=P))
            nc.sync.dma_start(out=bk_t[:].rearrange("p i -> p i ()"),
                              in_=bkd[:].rearrange("(i p) o -> p i o", p=P))
            nc.sync.dma_start(out=bq_t[:].rearrange("p i -> p i ()"),
                              in_=bqd[:].rearrange("(i p) o -> p i o", p=P))
            nc.sync.dma_start(
                out=wqr[:, :, P:DL],
                in_=wqT[:, P:DL].rearrange("(e p) d -> p e d", p=P))
            nc.scalar.dma_start(
                out=xqc0[:].rearrange("p (e s) -> p e s", e=ET)[:, :, P:QC],
                in_=xqT[:, P:QC].rearrange("(e p) s -> p e s", p=P))
            # PE clock warm-up while the first loads are in flight: harmless
            # zero matmuls keep the PE busy so it reaches full p-state before
            # the first projection.
            for wu in range(26):
                wsc = spool.tile([P, KG * P], f32, tag="sc", name="sc")
                for j in range(4):
                    mm(wsc[:, j * P : (j + 1) * P], warm[:], warm[:], True, True)

            # exp-engine policy: offload a few groups per slot to DVE/gpsimd
            # so the ACT stream never paces the kernel. Slot 0 is PE-bound
            # (projections); steady slots get 4/16 offloaded; the last slot
            # drains across all three engines to shorten the tail.
            # gpsimd cannot touch PSUM on HW, so only ACT and DVE share exp
            for kg in range(1, NKG):
                for h in (2, 3):
                    exp_eng[(0, h, kg)] = 1
            for qi in range(1, ST - 1):
                exp_eng[(qi, 0, 0)] = 1
                for h in range(1, H):
                    exp_eng[(qi, h, 1)] = 1
                for h in (1, 5):
                    exp_eng[(qi, h, 3)] = 1
            for h in range(H):
                for kg in range(NKG):
                    exp_eng[(ST - 1, h, kg)] = (4 * h + kg) % 2

            budget = [0.0]

            def drain(inter, force=False, to_marker=None):
                while inter:
                    if (not force and to_marker is None
                            and piece_cost(inter[0]) > budget[0]):
                        return
                    p = inter.popleft()
                    run_piece(p)
                    budget[0] -= piece_cost(p)
                    if to_marker is not None and p == ("HPM", to_marker):
                        return

            def do_sc(qi, h, kg, inter):
                sc_use(qi, h, kg)
                budget[0] = min(budget[0] + (EXP_NS - SC_NS), 3200.0)
                drain(inter)

            # slot 0: interleave K/Q chunk-0 projections with the first score
            # groups (head pair hp becomes ready as soon as dl-tile hp
            # projects), then kg 1-3 as K chunks 1-3 land.
            inter0 = deque(slot_inter[0])
            inter0.extendleft(reversed(
                [(qproj_rest, 0, dch) for dch in range(NDT)]))
            proj_piece(0, 0, wk, "xk", bk_t, kt)
            proj_piece(0, 1, wk, "xk", bk_t, kt)
            for dch in range(NDT):
                qproj_mini(0, dch)
                sc_use(0, 2 * dch, 0)
                sc_use(0, 2 * dch + 1, 0)
                if dch + 2 < NDT:
                    proj_piece(0, dch + 2, wk, "xk", bk_t, kt)
            xk_next = {}
            load_xk(1)
            xk_next[1] = vstate["xk"]
            for kg in range(1, NKG):
                vstate["xk"] = xk_next[kg]
                if kg + 1 < NKG:
                    load_xk(kg + 1)
                    xk_next[kg + 1] = vstate["xk"]
                    vstate["xk"] = xk_next[kg]
                for dch in range(NDT):
                    proj_piece(kg, dch, wk, "xk", bk_t, kt)
                for h in range(H):
                    do_sc(0, h, kg, inter0)
            drain(inter0, force=True)
            s1.close()
            open_vx()

            for qi in range(1, ST):
                inter = deque(slot_inter[qi])
                last = qi == ST - 1
                for h in range(H):
                    hp = h // 2
                    if h % 2 == 0:
                        if hp in slot_markers[qi]:
                            drain(inter, to_marker=hp)
                        for p in slot_hp[qi][hp]:
                            run_piece(p)
                    for kg in range(NKG):
                        if h == H - 1 and kg == 2:
                            # flush leftovers while the previous exp groups
                            # still cover the scalar engine
                            drain(inter, force=True)
                        do_sc(qi, h, kg, inter)
                    if last and h >= 1:
                        # final slot: attn@V runs one head behind the exp
                        # wave so the tail only holds the last head's chain
                        for kg in range(NKG):
                            run_piece((attnv_piece, qi, h - 1, kg))
                        if h - 1 == 3:
                            run_piece((norm_piece, qi, 0))
                            transp_qt(qi, half=0, via_pe=True)
                drain(inter, force=True)

            # ---------------- tail ----------------------------------------
            for kg in range(NKG):
                attnv_piece(ST - 1, H - 1, kg)
            norm_piece(ST - 1, 1)
            transp_qt(ST - 1, half=1, via_pe=True)
            for st in range(ST - 2, ST):
                for fc in range(D // QC):
                    c_piece(st, fc, copy_eng=nc.scalar)
            s3.close()

    nc.compile()
    return nc


def make_in_maps(query, key, value, Wq, bq, Wk, bk, Wv, bv, n_cores=8,
                 mm_dtype="float32r"):
    """Host-side sharding: slice weights Megatron-style, transpose activations."""
    import ml_dtypes

    bft = ml_dtypes.bfloat16
    q = np.asarray(query, dtype=np.float32)
    k = np.asarray(key, dtype=np.float32)
    v = np.asarray(value, dtype=np.float32)
    Wq = np.asarray(Wq, dtype=np.float32)
    Wk = np.asarray(Wk, dtype=np.float32)
    Wv = np.asarray(Wv, dtype=np.float32)
    bq = np.asarray(bq, dtype=np.float32)
    bk = np.asarray(bk, dtype=np.float32)
    D = Wq.shape[0]
    DL = D // (n_cores // q.shape[0])
    scale = np.float32(CQK)
    in_maps = []
    for c in range(n_cores):
        b, g = divmod(c, n_cores // q.shape[0])
        sl = slice(DL * g, DL * (g + 1))
        in_maps.append(
            {
                "xqT": np.ascontiguousarray(q[b].T).astype(bft),
                "xkT": np.ascontiguousarray(k[b].T).astype(bft),
                "xvT": np.ascontiguousarray(v[b].T).astype(bft),
                "wqT": (np.ascontiguousarray(Wq[sl].T) * scale).astype(bft),
                "wkT": (np.ascontiguousarray(Wk[sl].T) * scale).astype(bft),
                "wvT": np.ascontiguousarray(Wv[sl].T).astype(bft),
                "bq": np.ascontiguousarray((bq[sl] * scale).reshape(DL, 1)),
                "bk": np.ascontiguousarray((bk[sl] * scale).reshape(DL, 1)),
            }
        )
    return in_maps


def add_wo_maps(in_maps, Wo, n_cores=8, n_batch=4, mm_dtype="float32r"):
    import ml_dtypes

    Wo = np.asarray(Wo, dtype=np.float32)
    D = Wo.shape[0]
    DL = D // (n_cores // n_batch)
    for c in range(n_cores):
        _, g = divmod(c, n_cores // n_batch)
        sl = slice(DL * g, DL * (g + 1))
        in_maps[c]["woT"] = np.ascontiguousarray(Wo[:, sl].T).astype(ml_dtypes.bfloat16)
    return in_maps


MM_DTYPE = "float32r"


def kernel(query, key, value, Wq, bq, Wk, bk, Wv, bv, Wo, bo):
    if "nc" not in _CACHE:
        _CACHE["nc"] = build_nc(mm_dtype=MM_DTYPE)
    nc = _CACHE["nc"]
    n_cores = 8
    in_maps = make_in_maps(
        query, key, value, Wq, bq, Wk, bk, Wv, bv, n_cores, MM_DTYPE
    )
    add_wo_maps(in_maps, Wo, n_cores, np.asarray(query).shape[0], MM_DTYPE)
    bo = np.asarray(bo, dtype=np.float32)
    bv = np.asarray(bv, dtype=np.float32)
    Wo = np.asarray(Wo, dtype=np.float32)
    const = bo + bv @ Wo.T
    out = None
    for _attempt in range(3):
        res = run_bass_kernel_spmd(nc, in_maps, list(range(n_cores)))
        ys = [res.results[c]["y"] for c in range(n_cores)]
        out = (np.stack([ys[2 * b] + ys[2 * b + 1] for b in range(4)])
               + const[None, None, :])
        # the very first dispatch through the device tunnel occasionally
        # returns garbage; inputs are finite so a non-finite output means
        # the run was bad -- retry.
        if np.isfinite(out).all():
            break
    return out.astype(np.float32)



# revision 103
# speedup vs baseline: 1.0018x; 1.0018x over previous
"""Trainium2 Bass kernel for nn_MultiHeadAttention_37838661877847.

Full-input contract: kernel(**inputs) takes the complete tensors and returns
the complete output. Internally shards across 8 NeuronCores:
  core c -> batch b = c // 2, head-group g = c % 2 (8 heads, 512 dims each).
Each core computes Q/K/V projections for its (batch, head-group) slice
(column-parallel weights), attention for its 8 heads, and a partial output
projection (row-parallel Wo). Host sums core pairs and adds bo + bv @ Wo.T
(the V bias commutes through softmax-weighted averaging, so it is folded
into the output-projection bias on the host).

Engine-level design (per core), built as ONE interleaved instruction stream.
The softmax exp (33.5M elems/core, a ~266us serial floor on the ACT engine
alone) is SPLIT between the ACT engine (true exp, ~2/3 of tiles) and the DVE
(Schraudolph bit-trick exp, ~1/3), so the tensor engine (~283us busy) paces
the kernel instead of the exp stream:

  - Q_T/K_T stored (dl, s) in bf16, host-scaled by CQK each so the score
    PSUM equals 128*log2(e)*s_true. ACT tiles: exp via activation scale;
    DVE tiles: probs bf16 bits = trunc(psum + SCHRAUD_B) written as int16
    through a bitcast AP (~1.8% multiplicative noise, mean-calibrated).
  - scores come out (k, q) per 128-k tile; one [128,512] PSUM bank per
    (head, k-group) so FOUR groups pipeline in flight across the two exp
    engines (spool bufs=4); ets tiles in bf16.
  - attn@V is FLIPPED: out (q, dk+1) accumulating over k with the exp tile
    as the stationary operand -> 65-row bf16 matmuls. V is augmented with a
    ones column per head so the softmax denominator Z lands in column 64;
    normalization is a per-partition reciprocal+scale on DVE.
  - normalized output (q, dl) in bf16 is transposed back to (dl, q) via
    bf16 PE-transpose (bf16 identity: 1 cycle/row), then the output
    projection streams wo.
  - emission interleaves projections / attn@V / transposes / out-proj
    between score+exp groups under a PE-lead budget; startup splits the
    first Q-side DMAs so the serialized DMA resource feeds K then Q with
    minimal PE idle.
"""

import sys

sys.path.insert(0, "/opt/trn_rl_repo")

from collections import deque
from contextlib import ExitStack

import numpy as np

import concourse.bass as bass  # noqa: F401
import concourse.tile as tile
from concourse import bacc, masks, mybir
from concourse.bass_utils import run_bass_kernel_spmd

P = 128
DK = 64  # head dim

# Softmax slope-1 trick: host scales Wq/Wk by CQK each so the score PSUM is
# exactly 128*log2(e) * s_true. The ACT engine then exps with scale=1/SLOPE,
# while DVE/gpsimd compute the same softmax numerator via the Schraudolph
# bit trick: bf16_bits(exp(s)) ~= trunc(psum + SCHRAUD_B) as int16. This
# lets all three engines share the exp workload.
SLOPE = 184.66505644  # 128 / ln(2)
CQK = 4.8044902       # sqrt(SLOPE / 8); 8 = sqrt(dk)
ACT_SCALE = 1.0 / SLOPE
SCHRAUD_B = 16249.0   # 16256 (bf16 bits of 1.0) - 7 mean-ratio calibration

_CACHE = {}


def build_nc(S=2048, D=1024, DL=512, mm_dtype="float32r", n_cores=8,
             repeats=1, phases="ABC"):
    """Build + compile the per-core Bass program (same program on all cores).

    repeats exists only for timing experiments; production uses the default.
    mm_dtype/phases are accepted for test-harness compatibility (the kernel
    uses a fixed mixed f32r/bf16 precision scheme).
    """
    f32 = mybir.dt.float32
    f32r = mybir.dt.float32r
    bf16 = mybir.dt.bfloat16
    Exp = mybir.ActivationFunctionType.Exp

    H = DL // DK          # 8 local heads
    HP = H // 2           # 4 head pairs (one pair per 128-row q/k tile)
    ET = D // P           # 8 contraction tiles for projections
    ST = S // P           # 16 k tiles (and q tiles)
    NDT = DL // P         # 4 dl tiles
    QC = 512              # projection s-chunk
    NQ = S // QC          # 4
    KG = 4                # k-tiles per exp group
    NKG = ST // KG        # 4
    VW = H * (DK + 1)     # 520: v tile width incl. ones columns

    nc = bacc.Bacc("TRN2", target_bir_lowering=False, num_devices=n_cores)

    xqT = nc.dram_tensor("xqT", [D, S], bf16, kind="ExternalInput")
    xkT = nc.dram_tensor("xkT", [D, S], bf16, kind="ExternalInput")
    xvT = nc.dram_tensor("xvT", [D, S], bf16, kind="ExternalInput")
    wqT = nc.dram_tensor("wqT", [D, DL], bf16, kind="ExternalInput")
    wkT = nc.dram_tensor("wkT", [D, DL], bf16, kind="ExternalInput")
    wvT = nc.dram_tensor("wvT", [D, DL], bf16, kind="ExternalInput")
    woT = nc.dram_tensor("woT", [DL, D], bf16, kind="ExternalInput")
    bqd = nc.dram_tensor("bq", [DL, 1], f32, kind="ExternalInput")
    bkd = nc.dram_tensor("bk", [DL, 1], f32, kind="ExternalInput")
    y = nc.dram_tensor("y", [S, D], f32, kind="ExternalOutput")

    def mm(out, lhsT, rhs, start, stop):
        nc.tensor.matmul(out, lhsT=lhsT, rhs=rhs, start=start, stop=stop)

    with tile.TileContext(nc) as tc, ExitStack() as top:
        top.enter_context(
            nc.allow_low_precision(
                reason="attention path in bf16; PSUM accumulation stays fp32"
            )
        )
        persist = top.enter_context(tc.tile_pool(name="persist", bufs=1))
        qt = [persist.tile([P, S], bf16, tag=f"qt{i}", name=f"qt{i}") for i in range(NDT)]
        kt = [persist.tile([P, S], bf16, tag=f"kt{i}", name=f"kt{i}") for i in range(NDT)]
        vt = [persist.tile([P, VW], bf16, tag=f"vt{i}", name=f"vt{i}") for i in range(ST)]
        oaT = [persist.tile([P, S], bf16, tag=f"oaT{i}", name=f"oaT{i}") for i in range(NDT)]
        ident = persist.tile([P, P], bf16, tag="ident", name="ident")
        bq_t = persist.tile([P, NDT], f32, tag="bq", name="bq")
        bk_t = persist.tile([P, NDT], f32, tag="bk", name="bk")

        masks.make_identity(nc, ident[:])
        warm = persist.tile([P, P], bf16, tag="warm", name="warm")
        nc.vector.memset(warm[:], 0.0)
        # vt ones-columns are memset inside slot 0 (below) so the head's
        # K/Q projection evacuations reach the DVE queue first.

        # PSUM: scores/exp 4x[128,512] (4 banks) + attn@V accum 2x[128,260]
        # (2 banks) + generic matmul 2x[128,512] (2 banks) = 8 banks.
        spool = top.enter_context(tc.tile_pool(name="spool", bufs=4, space="PSUM"))
        acpool = top.enter_context(tc.tile_pool(name="acpool", bufs=2, space="PSUM"))
        gpool = top.enter_context(tc.tile_pool(name="gpool", bufs=2, space="PSUM"))

        # weight/x pools for Q (live through all Q chunks); wide layouts:
        # w tiles hold all ET contraction blocks side by side (one DMA each).
        wqp = top.enter_context(tc.tile_pool(name="wqp", bufs=1))
        wq = wqp.tile([P, ET * DL], bf16, tag="wq", name="wq")
        xqp = top.enter_context(tc.tile_pool(name="xqp", bufs=1))

        # long-lived attention pools (opened before any scoped pool so that
        # mid-stream pool closes stay LIFO)
        etsp = top.enter_context(tc.tile_pool(name="etsp", bufs=2))
        oasp = top.enter_context(tc.tile_pool(name="oasp", bufs=3))
        yvp = top.enter_context(tc.tile_pool(name="yvp", bufs=3))
        rcp = top.enter_context(tc.tile_pool(name="rcp", bufs=4))

        for _rep in range(repeats):
            # ---------------- pools for K and Q chunk streams -------------
            vstate = {}
            s3 = ExitStack()
            s2 = ExitStack()
            vxa = s2.enter_context(tc.tile_pool(name="vxa", bufs=1))
            vstate["wv"] = vxa.tile([P, ET * DL], bf16, tag="wv", name="wv")
            vstate["xv0"] = vxa.tile([P, ET * (S // 2)], bf16, tag="xv0",
                                     name="xv0")
            s1 = ExitStack()
            kx = s1.enter_context(tc.tile_pool(name="kx", bufs=2))
            wkp = s1.enter_context(tc.tile_pool(name="wkp", bufs=1))
            wk = wkp.tile([P, ET * DL], bf16, tag="wk", name="wk")

            def load_xk(c, eng=None):
                xkc = kx.tile([P, ET * QC], bf16, tag="xk", name="xk")
                (eng or nc.sync).dma_start(
                    out=xkc[:].rearrange("p (e s) -> p e s", e=ET),
                    in_=xkT[:, c * QC : (c + 1) * QC].rearrange(
                        "(e p) s -> p e s", p=P),
                )
                vstate["xk"] = xkc

            def load_xq(c, eng):
                xqc = xqp.tile([P, ET * QC], bf16, tag="xq", name="xq")
                eng.dma_start(
                    out=xqc[:].rearrange("p (e s) -> p e s", e=ET),
                    in_=xqT[:, c * QC : (c + 1) * QC].rearrange(
                        "(e p) s -> p e s", p=P),
                )
                vstate["xq"] = xqc

            def proj_piece(c, dch, w, xkey, bias, out_tiles):
                """One (chunk, dl-tile) projection: out (dl 128, s 512) + bias."""
                x = vstate[xkey]
                gp = gpool.tile([P, QC], f32, tag="gp", name="gp")
                for e in range(ET):
                    mm(gp[:], w[:, e * DL + dch * P : e * DL + (dch + 1) * P],
                       x[:, e * QC : (e + 1) * QC], e == 0, e == ET - 1)
                nc.vector.tensor_scalar_add(
                    out_tiles[dch][:, c * QC : (c + 1) * QC], gp[:],
                    bias[:, dch : dch + 1]
                )

            # --- V pools: wv + the first s-half of xv preload alongside the
            # K pool (slot 0); the second s-half lands in the space the K pool
            # frees. V projection runs head-half-major so attn@V for heads 0-3
            # unblocks as early as possible.
            SH = S // 2

            def load_wv():
                nc.sync.dma_start(
                    out=vstate["wv"][:].rearrange("p (e d) -> p e d", e=ET),
                    in_=wvT[:].rearrange("(e p) d -> p e d", p=P),
                )

            def load_xv0():
                nc.sync.dma_start(
                    out=vstate["xv0"][:].rearrange("p (e s) -> p e s", e=ET),
                    in_=xvT[:, 0:SH].rearrange("(e p) s -> p e s", p=P),
                )

            def open_vx():
                vxb = s2.enter_context(tc.tile_pool(name="vxb", bufs=1))
                vstate["xv1"] = vxb.tile([P, ET * SH], bf16, tag="xv1", name="xv1")
                nc.sync.dma_start(
                    out=vstate["xv1"][:].rearrange("p (e s) -> p e s", e=ET),
                    in_=xvT[:, SH:S].rearrange("(e p) s -> p e s", p=P),
                )

            def vproj_piece(st, qtr):
                """V projection for (s-tile st, head pair qtr): 2 heads.
                Quarters 0-1 run before attn@V starts, so odd s-tiles borrow
                the idle attn@V accumulator banks for deeper pipelining."""
                Q4 = DL // 4
                if qtr < 2 and st % 2 == 1:
                    gp = acpool.tile([P, 512], f32, tag="ac", name="ac")
                else:
                    gp = gpool.tile([P, QC], f32, tag="gp", name="gp")
                wv = vstate["wv"]
                xv = vstate["xv0"] if st < ST // 2 else vstate["xv1"]
                stl = st % (ST // 2)
                for e in range(ET):
                    mm(gp[:, 0:Q4],
                       xv[:, e * SH + stl * P : e * SH + (stl + 1) * P],
                       wv[:, e * DL + qtr * Q4 : e * DL + (qtr + 1) * Q4],
                       e == 0, e == ET - 1)
                nc.vector.tensor_copy(
                    vt[st][:].rearrange("p (h c) -> p h c", h=H)
                    [:, qtr * 2 : (qtr + 1) * 2, 0:DK],
                    gp[:, 0:Q4].rearrange("p (h c) -> p h c", h=2),
                )

            state = {
                "ets": {},    # (qt_idx, hp, kg) -> tile  (live window)
                "ac": {},     # (qt_idx, hgrp) -> tile
                "oas": {},    # qt_idx -> tile
                "wo": None,
            }

            exp_eng = {}  # (qi, h, kg) -> 0 act / 1 dve / 2 pool

            def sc_use(qi, h, kg):
                """Scores + exp for (q-tile qi, head h, k-group kg).

                One PSUM bank per group so four groups pipeline in flight;
                the exp may run on ACT (true exp) or DVE/gpsimd (Schraudolph
                bits via int16 add, see module docstring constants)."""
                hp, r0 = divmod(h, 2)
                r0 *= DK
                sc = spool.tile([P, KG * P], f32, tag="sc", name="sc")
                for ktl in range(KG):
                    ki = kg * KG + ktl
                    mm(
                        sc[:, ktl * P : (ktl + 1) * P],
                        kt[hp][r0 : r0 + DK, ki * P : (ki + 1) * P],
                        qt[hp][r0 : r0 + DK, qi * P : (qi + 1) * P],
                        True,
                        True,
                    )
                et = etsp.tile([P, KG * P], bf16, tag=f"et{h}_{kg}",
                               name=f"et{h}_{kg}")
                eng = exp_eng.get((qi, h, kg), 0)
                if eng == 0:
                    nc.scalar.activation(et[:], sc[:], Exp, scale=ACT_SCALE)
                else:
                    eo = nc.vector if eng == 1 else nc.gpsimd
                    eo.tensor_scalar_add(
                        et[:].bitcast(mybir.dt.int16), sc[:], SCHRAUD_B
                    )
                state["ets"][(qi, h, kg)] = et

            def attnv_piece(qi, h, kg):
                """attn@V for (q-tile qi, head h, k-group kg): 4 x 65-row mms."""
                hgrp, hidx = divmod(h, 4)
                key = (qi, hgrp)
                if key not in state["ac"]:
                    # padded to a full 2KB bank; cols 0-259 used (4 heads x 65)
                    state["ac"][key] = acpool.tile([P, 512], f32, tag="ac", name="ac")
                ac = state["ac"][key]
                et = state["ets"][(qi, h, kg)]
                for ktl in range(KG):
                    ki = kg * KG + ktl
                    mm(
                        ac[:, hidx * (DK + 1) : (hidx + 1) * (DK + 1)],
                        et[:, ktl * P : (ktl + 1) * P],
                        vt[ki][:, h * (DK + 1) : (h + 1) * (DK + 1)],
                        ki == 0,
                        ki == ST - 1,
                    )
                del state["ets"][(qi, h, kg)]

            def norm_piece(qi, hgrp, on_act=False):
                """Normalize 4 heads: oa_s[:, hgrp*256:+256] = num * (1/Z).
                on_act moves the scale-muls to the ACT engine (used in the
                tail where ACT is idle but DVE still drains exps)."""
                if qi not in state["oas"]:
                    state["oas"][qi] = oasp.tile([P, DL], bf16, tag="oas", name="oas")
                oas = state["oas"][qi]
                ac = state["ac"].pop((qi, hgrp))
                acr = ac[:, 0 : 4 * (DK + 1)].rearrange("p (h c) -> p h c", h=4)
                rc = rcp.tile([P, 4], f32, tag="rc", name="rc")
                nc.vector.reciprocal(rc[:], acr[:, :, DK])
                for hh in range(4):
                    dst = oas[:, hgrp * 4 * DK + hh * DK
                              : hgrp * 4 * DK + (hh + 1) * DK]
                    if on_act:
                        nc.scalar.mul(dst, acr[:, hh, 0:DK], rc[:, hh : hh + 1])
                    else:
                        nc.vector.tensor_scalar_mul(
                            dst, acr[:, hh, 0:DK], rc[:, hh : hh + 1])

            def transp_qt(qi, half=None, via_pe=True, copy_act=False):
                """Transpose oa_s (q, dl) -> oaT (dl, q) for one q-tile.
                half=0 covers dl-tiles 0-1 (ready after the hgrp-0 norm),
                half=1 covers 2-3; None does both. Steady-state q-tiles use
                the DMA xbar transpose (off the PE/DVE critical engines;
                c_piece consumes oaT two slots later so the ~2.5us DMA
                latency is free); the final q-tile stays on the PE path to
                keep the tail chain short."""
                rng = (range(NDT) if half is None
                       else range(2 * half, 2 * half + 2))
                oas = state["oas"][qi]
                if not via_pe:
                    for dlb in rng:
                        nc.sync.dma_start_transpose(
                            oaT[dlb][:, qi * P : (qi + 1) * P],
                            oas[:, dlb * P : (dlb + 1) * P],
                        )
                else:
                    sc = acpool.tile([P, 512], bf16, tag="ac", name="ac")
                    for dlb in rng:
                        nc.tensor.transpose(
                            sc[:, dlb * P : (dlb + 1) * P],
                            oas[:, dlb * P : (dlb + 1) * P],
                            ident[:],
                        )
                    for dlb in rng:
                        (nc.scalar.copy if copy_act else nc.vector.tensor_copy)(
                            oaT[dlb][:, qi * P : (qi + 1) * P],
                            sc[:, dlb * P : (dlb + 1) * P],
                        )
                if half != 0:
                    del state["oas"][qi]

            def load_wo():
                wop = s3.enter_context(tc.tile_pool(name="wop", bufs=1))
                wo = wop.tile([P, NDT * D], bf16, tag="wo", name="wo")
                nc.gpsimd.dma_start(
                    out=wo[:].rearrange("p (i d) -> p i d", i=NDT),
                    in_=woT[:].rearrange("(i p) d -> p i d", p=P),
                )
                state["wo"] = wo

            def c_piece(st, fc, copy_eng=None):
                """Output projection for (s-tile st, f-chunk fc). The PSUM
                evacuation runs on DVE mid-stream; the tail passes the ACT
                engine (idle by then) so the last stores don't queue behind
                DVE's final exp wave."""
                wo = state["wo"]
                gp = gpool.tile([P, QC], f32, tag="gp", name="gp")
                for dl in range(NDT):
                    mm(gp[:], oaT[dl][:, st * P : (st + 1) * P],
                       wo[:, dl * D + fc * QC : dl * D + (fc + 1) * QC],
                       dl == 0, dl == NDT - 1)
                yv = yvp.tile([P, QC], f32, tag="yv", name="yv")
                if copy_eng is None:
                    nc.vector.tensor_copy(yv[:], gp[:])
                else:
                    copy_eng.copy(yv[:], gp[:])
                nc.sync.dma_start(
                    out=y[st * P : (st + 1) * P, fc * QC : (fc + 1) * QC], in_=yv[:]
                )

            # ------------- interleaved emission ---------------------------
            # One FIFO of side pieces per slot, drained between score+exp
            # groups under a PE-lead budget, force-drained at slot end (and at
            # the MID marker before the hp2/hp3 half). Estimated PE ns/piece.
            EXP_NS, SC_NS = 610.0, 215.0
            COST = {}

            def piece_cost(p):
                fn = p[0]
                if fn == proj_piece:
                    return 1750.0
                if fn == qproj_rest:
                    return 1350.0
                if fn == vproj_piece:
                    return 450.0
                if fn == attnv_piece:
                    return 160.0
                if fn == transp_qt:
                    return 520.0
                if fn == c_piece:
                    return 900.0
                return 0.0

            def run_piece(p):
                if p[0] == "loadxk":
                    load_xk(p[1])
                elif p[0] == "loadxq":
                    load_xq(p[1], nc.gpsimd)
                elif p[0] == "loadwo":
                    load_wo()
                elif p[0] == "closes1":
                    s1.close()
                elif p[0] == "openvx":
                    open_vx()
                elif p[0] == "memset":
                    # only the per-head ones-columns (col DK of each head)
                    nc.vector.memset(
                        vt[p[1]][:].rearrange("p (h c) -> p h c", h=H)[:, :, DK],
                        1.0,
                    )
                elif p[0] == "loadwv":
                    load_wv()
                elif p[0] == "loadxv0":
                    load_xv0()
                elif p[0] == "closes2":
                    s2.close()
                elif p[0] == "HPM":
                    pass
                else:
                    p[0](*p[1:])

            def attnv_hp(qi, hp):
                out = []
                for h in (2 * hp, 2 * hp + 1):
                    for kg in range(NKG):
                        out.append((attnv_piece, qi, h, kg))
                return out

            slot_inter = [[] for _ in range(ST)]
            slot_markers = [set() for _ in range(ST)]
            slot_inter[0] += [("memset", i) for i in range(ST)]
            slot_inter[0] += [("loadwv",), ("loadxv0",)]
            slot_inter[0] += [(vproj_piece, st, 0) for st in range(ST // 2)]
            slot_inter[1] += [(vproj_piece, st, 0) for st in range(ST // 2, ST)]
            slot_inter[1] += attnv_hp(0, 0)
            slot_inter[1] += [(vproj_piece, st, 1) for st in range(ST)]
            slot_inter[1] += attnv_hp(0, 1) + [(norm_piece, 0, 0)]
            slot_inter[2] += [(vproj_piece, st, 2) for st in range(ST)]
            slot_inter[2] += attnv_hp(0, 2) + [("HPM", 2)]
            slot_inter[2] += [(vproj_piece, st, 3) for st in range(ST)]
            slot_inter[2] += attnv_hp(0, 3) + [(norm_piece, 0, 1), ("HPM", 3)]
            slot_inter[2] += [("closes2",), ("loadwo",)]
            slot_markers[2] = {2, 3}
            slot_inter[3] += (attnv_hp(1, 0) + [("HPM", 0)]
                              + attnv_hp(1, 1) + [(norm_piece, 1, 0), ("HPM", 1)]
                              + attnv_hp(1, 2) + [("HPM", 2)]
                              + attnv_hp(1, 3) + [(norm_piece, 1, 1), ("HPM", 3)])
            slot_markers[3] = {0, 1, 2, 3}
            slot_inter[3] += [(transp_qt, 0), (transp_qt, 1)]
            for j in range(3, ST):
                slot_inter[j] += (attnv_hp(j - 1, 0) + attnv_hp(j - 1, 1)
                                  + [(norm_piece, j - 1, 0)]
                                  + attnv_hp(j - 1, 2) + attnv_hp(j - 1, 3)
                                  + [(norm_piece, j - 1, 1)])
                if j - 1 >= 2:
                    slot_inter[j].append((transp_qt, j - 1))
            # out-projection: early s-tiles deferred to late slots (the early
            # slots carry the V/K/Q overload), the rest two slots after their
            # transpose.
            for st in range(0, 5):
                slot_inter[11 + st] += [(c_piece, st, 0), (c_piece, st, 1)]
            for st in range(5, 14):
                slot_inter[st + 2] += [(c_piece, st, 0), (c_piece, st, 1)]
            # Q chunk c: DMA early, project each dl-tile just before the
            # first score group of slot 4c that needs it.
            def qproj_mini(c, dch):
                """Q projection for q-tile 4c only (slot 4c's own columns)."""
                x = vstate["xq"]
                gp = gpool.tile([P, QC], f32, tag="gp", name="gp")
                for e in range(ET):
                    mm(gp[:, 0:P],
                       wq[:, e * DL + dch * P : e * DL + (dch + 1) * P],
                       x[:, e * QC : e * QC + P], e == 0, e == ET - 1)
                nc.vector.tensor_scalar_add(
                    qt[dch][:, 4 * c * P : (4 * c + 1) * P], gp[:, 0:P],
                    bq_t[:, dch : dch + 1])

            def qproj_rest(c, dch):
                """Q projection for q-tiles 4c+1..4c+3 (needed next slot)."""
                x = vstate["xq"]
                gp = gpool.tile([P, QC], f32, tag="gp", name="gp")
                for e in range(ET):
                    mm(gp[:, 0 : 3 * P],
                       wq[:, e * DL + dch * P : e * DL + (dch + 1) * P],
                       x[:, e * QC + P : (e + 1) * QC], e == 0, e == ET - 1)
                nc.vector.tensor_scalar_add(
                    qt[dch][:, (4 * c + 1) * P : (4 * c + 4) * P],
                    gp[:, 0 : 3 * P], bq_t[:, dch : dch + 1])

            slot_hp = [[[] for _ in range(HP)] for _ in range(ST)]
            for c in (1, 2, 3):
                slot_inter[4 * c - 3].append(("loadxq", c))
                for dch in range(NDT):
                    slot_hp[4 * c][dch].append((qproj_mini, c, dch))
                slot_inter[4 * c] += [(qproj_rest, c, dch)
                                      for dch in range(NDT)]

            # ---------------- head + slot loop ----------------------------
            # initial loads: tiny biases first, then the four big transfers in
            # the order the serialized DMA resource should grant them — wk and
            # xk0 feed the first K projections; the Q side is split so q-tile-0
            # columns (xq0a, wq dch-0) land early enough for the first scores.
            nc.scalar.dma_start(
                out=wk[:].rearrange("p (e d) -> p e d", e=ET),
                in_=wkT[:].rearrange("(e p) d -> p e d", p=P))
            load_xk(0)
            xqc0 = xqp.tile([P, ET * QC], bf16, tag="xq", name="xq")
            vstate["xq"] = xqc0
            nc.scalar.dma_start(
                out=xqc0[:].rearrange("p (e s) -> p e s", e=ET)[:, :, 0:P],
                in_=xqT[:, 0:P].rearrange("(e p) s -> p e s", p=P))
            wqr = wq[:].rearrange("p (e d) -> p e d", e=ET)
            nc.sync.dma_start(
                out=wqr[:, :, 0:P],
                in_=wqT[:, 0:P].rearrange("(e p) d -> p e d", p=P))
            nc.sync.dma_start(out=bk_t[:].rearrange("p i -> p i ()"),
                              in_=bkd[:].rearrange("(i p) o -> p i o", p=P))
            nc.sync.dma_start(out=bq_t[:].rearrange("p i -> p i ()"),
                              in_=bqd[:].rearrange("(i p) o -> p i o", p=P))
            nc.sync.dma_start(
                out=wqr[:, :, P:DL],
                in_=wqT[:, P:DL].rearrange("(e p) d -> p e d", p=P))
            nc.scalar.dma_start(
                out=xqc0[:].rearrange("p (e s) -> p e s", e=ET)[:, :, P:QC],
                in_=xqT[:, P:QC].rearrange("(e p) s -> p e s", p=P))
            # PE clock warm-up while the first loads are in flight: harmless
            # zero matmuls keep the PE busy so it reaches full p-state before
            # the first projection.
            for wu in range(26):
                wsc = spool.tile([P, KG * P], f32, tag="sc", name="sc")
                for j in range(4):
                    mm(wsc[:, j * P : (j + 1) * P], warm[:], warm[:], True, True)

            # exp-engine policy: offload a few groups per slot to DVE/gpsimd
            # so the ACT stream never paces the kernel. Slot 0 is PE-bound
            # (projections); steady slots get 4/16 offloaded; the last slot
            # drains across all three engines to shorten the tail.
            # gpsimd cannot touch PSUM on HW, so only ACT and DVE share exp
            for kg in range(1, NKG):
                for h in (2, 3):
                    exp_eng[(0, h, kg)] = 1
            for qi in range(1, ST - 1):
                exp_eng[(qi, 0, 0)] = 1
                for h in range(1, H):
                    exp_eng[(qi, h, 1)] = 1
                for h in (1, 5):
                    exp_eng[(qi, h, 3)] = 1
            for h in range(H):
                for kg in range(NKG):
                    exp_eng[(ST - 1, h, kg)] = 1 - (4 * h + kg) % 2

            budget = [0.0]

            def drain(inter, force=False, to_marker=None):
                while inter:
                    if (not force and to_marker is None
                            and piece_cost(inter[0]) > budget[0]):
                        return
                    p = inter.popleft()
                    run_piece(p)
                    budget[0] -= piece_cost(p)
                    if to_marker is not None and p == ("HPM", to_marker):
                        return

            def do_sc(qi, h, kg, inter):
                sc_use(qi, h, kg)
                budget[0] = min(budget[0] + (EXP_NS - SC_NS), 3200.0)
                drain(inter)

            # slot 0: interleave K/Q chunk-0 projections with the first score
            # groups (head pair hp becomes ready as soon as dl-tile hp
            # projects), then kg 1-3 as K chunks 1-3 land.
            inter0 = deque(slot_inter[0])
            inter0.extendleft(reversed(
                [(qproj_rest, 0, dch) for dch in range(NDT)]))
            proj_piece(0, 0, wk, "xk", bk_t, kt)
            proj_piece(0, 1, wk, "xk", bk_t, kt)
            for dch in range(NDT):
                qproj_mini(0, dch)
                sc_use(0, 2 * dch, 0)
                sc_use(0, 2 * dch + 1, 0)
                if dch + 2 < NDT:
                    proj_piece(0, dch + 2, wk, "xk", bk_t, kt)
            xk_next = {}
            load_xk(1)
            xk_next[1] = vstate["xk"]
            for kg in range(1, NKG):
                vstate["xk"] = xk_next[kg]
                if kg + 1 < NKG:
                    load_xk(kg + 1)
                    xk_next[kg + 1] = vstate["xk"]
                    vstate["xk"] = xk_next[kg]
                for dch in range(NDT):
                    proj_piece(kg, dch, wk, "xk", bk_t, kt)
                for h in range(H):
                    do_sc(0, h, kg, inter0)
            drain(inter0, force=True)
            s1.close()
            open_vx()

            for qi in range(1, ST):
                inter = deque(slot_inter[qi])
                last = qi == ST - 1
                for h in range(H):
                    hp = h // 2
                    if h % 2 == 0:
                        if hp in slot_markers[qi]:
                            drain(inter, to_marker=hp)
                        for p in slot_hp[qi][hp]:
                            run_piece(p)
                    for kg in range(NKG):
                        if h == H - 1 and kg == 2:
                            # flush leftovers while the previous exp groups
                            # still cover the scalar engine
                            drain(inter, force=True)
                        do_sc(qi, h, kg, inter)
                    if last and h >= 1:
                        # final slot: attn@V runs one head behind the exp
                        # wave so the tail only holds the last head's chain
                        for kg in range(NKG):
                            run_piece((attnv_piece, qi, h - 1, kg))
                        if h - 1 == 3:
                            run_piece((norm_piece, qi, 0))
                            transp_qt(qi, half=0, via_pe=True)
                drain(inter, force=True)

            # ---------------- tail ----------------------------------------
            for kg in range(NKG):
                attnv_piece(ST - 1, H - 1, kg)
            norm_piece(ST - 1, 1)
            transp_qt(ST - 1, half=1, via_pe=True)
            for st in range(ST - 2, ST):
                for fc in range(D // QC):
                    c_piece(st, fc, copy_eng=nc.scalar)
            s3.close()

    nc.compile()
    return nc


def make_in_maps(query, key, value, Wq, bq, Wk, bk, Wv, bv, n_cores=8,
                 mm_dtype="float32r"):
    """Host-side sharding: slice weights Megatron-style, transpose activations."""
    import ml_dtypes

    bft = ml_dtypes.bfloat16
    q = np.asarray(query, dtype=np.float32)
    k = np.asarray(key, dtype=np.float32)
    v = np.asarray(value, dtype=np.float32)
    Wq = np.asarray(Wq, dtype=np.float32)
    Wk = np.asarray(Wk, dtype=np.float32)
    Wv = np.asarray(Wv, dtype=np.float32)
    bq = np.asarray(bq, dtype=np.float32)
    bk = np.asarray(bk, dtype=np.float32)
    D = Wq.shape[0]
    DL = D // (n_cores // q.shape[0])
    scale = np.float32(CQK)
    in_maps = []
    for c in range(n_cores):
        b, g = divmod(c, n_cores // q.shape[0])
        sl = slice(DL * g, DL * (g + 1))
        in_maps.append(
            {
                "xqT": np.ascontiguousarray(q[b].T).astype(bft),
                "xkT": np.ascontiguousarray(k[b].T).astype(bft),
                "xvT": np.ascontiguousarray(v[b].T).astype(bft),
                "wqT": (np.ascontiguousarray(Wq[sl].T) * scale).astype(bft),
                "wkT": (np.ascontiguousarray(Wk[sl].T) * scale).astype(bft),
                "wvT": np.ascontiguousarray(Wv[sl].T).astype(bft),
                "bq": np.ascontiguousarray((bq[sl] * scale).reshape(DL, 1)),
                "bk": np.ascontiguousarray((bk[sl] * scale).reshape(DL, 1)),
            }
        )
    return in_maps


def add_wo_maps(in_maps, Wo, n_cores=8, n_batch=4, mm_dtype="float32r"):
    import ml_dtypes

    Wo = np.asarray(Wo, dtype=np.float32)
    D = Wo.shape[0]
    DL = D // (n_cores // n_batch)
    for c in range(n_cores):
        _, g = divmod(c, n_cores // n_batch)
        sl = slice(DL * g, DL * (g + 1))
        in_maps[c]["woT"] = np.ascontiguousarray(Wo[:, sl].T).astype(ml_dtypes.bfloat16)
    return in_maps


MM_DTYPE = "float32r"


def kernel(query, key, value, Wq, bq, Wk, bk, Wv, bv, Wo, bo):
    if "nc" not in _CACHE:
        _CACHE["nc"] = build_nc(mm_dtype=MM_DTYPE)
    nc = _CACHE["nc"]
    n_cores = 8
    in_maps = make_in_maps(
        query, key, value, Wq, bq, Wk, bk, Wv, bv, n_cores, MM_DTYPE
    )
    add_wo_maps(in_maps, Wo, n_cores, np.asarray(query).shape[0], MM_DTYPE)
    bo = np.asarray(bo, dtype=np.float32)
    bv = np.asarray(bv, dtype=np.float32)
    Wo = np.asarray(Wo, dtype=np.float32)
    const = bo + bv @ Wo.T
    out = None
    for _attempt in range(3):
        res = run_bass_kernel_spmd(nc, in_maps, list(range(n_cores)))
        ys = [res.results[c]["y"] for c in range(n_cores)]
        out = (np.stack([ys[2 * b] + ys[2 * b + 1] for b in range(4)])
               + const[None, None, :])
        # the very first dispatch through the device tunnel occasionally
        # returns garbage; inputs are finite so a non-finite output means
        # the run was bad -- retry.
        if np.isfinite(out).all():
            break
    return out.astype(np.float32)

